# revision 1
# baseline (speedup 1.0000x reference)
"""Trainium2 Bass kernel for nn_Covariance.

Math: for Xs [B,T,F,2,M], the reference forms per-(b,t,f) upper-triangular
complex covariance entries and replaces them with their time-mean
(broadcast back over T).  Writing x_tf = (re||im) in R^16, every needed
quantity is an entry of the time-summed Gram matrix C_f = sum_t x_tf x_tf^T:

    re_part(i,j) = C[i, j]   + C[8+i, 8+j]
    im_part(i,j) = C[i, 8+j] - C[j, 8+i]

Device kernel: per frequency f, compute C_f via PE matmuls with the
T-contraction on the partition axis (PSUM accumulates the 4 chunks of
T=512).  The input is split on the host into bf16 hi/lo parts x = H + M
(M = bf16(x - H)); frequencies are processed in pairs with the packed
layout {H_f0|H_f1|M_f0|M_f1} (64 bf16 columns per pair), so each
(pair, t-chunk) is ONE ldweights (32 cols: H_f0|H_f1) + ONE matmul
(N=64), yielding blocks G1=H^T H and G2=H^T M for both frequencies.
Pairs are spread over the PE array's four 32-column strips (inferred
tile_position from the PSUM-out partition offset) and issued round-robin
across strips so each strip's LDWEIGHTS overlaps the other strips'
matmul streaming.  Host reconstructs C ~= G1 + G2 + G2^T (the dropped
M^T M term is ~2^-18 relative), does the tiny triu-gather, /T scaling,
and the (redundant) time-broadcast.

Sharding: batch-parallel, one batch element per NeuronCore (B == 8 cores).
Per core: read 16.8 MB (bf16 {H|M}), write 2.36 MB of Gram blocks.
"""

import numpy as np

_B, _T, _F, _M = 8, 512, 513, 8
_CH = 2 * _M            # 16 packed re/im channels
_ROWS = 2 * _CH         # 32 output rows per pair (two frequencies' channels)
_PW = 4 * _CH           # 64 packed {H|H|M|M} columns per frequency pair
_NP = (_F + 1) // 2     # 257 frequency pairs (F padded to 514)
_KC = _T // 128         # 4 chunks of the time axis (PSUM-accumulated)
_NCORES = 8
_NSTRIP = 4             # PE column strips (32 rows of PSUM each)
_SLOTS = 8              # pairs per strip per PSUM bank ([128, 512])
_PG = _NSTRIP * _SLOTS  # 32 pairs per PSUM bank tile
_NPG = (_NP + _PG - 1) // _PG   # 9 PSUM groups (last holds 1 pair)
_OSTG = 4               # PSUM groups per output staging tile
_GCOL = _SLOTS * _PW    # 512 gram columns per PSUM group
# progressive DMA slices in pairs (first covers PSUM group 0 exactly)
_SLICES = [(0, 32), (32, 64), (96, 64), (160, 64), (224, 33)]
_PSMAX = 64             # max pairs per slice (tile tag sizing)

_nc_cache = None


def _build_nc(reps=1, dma_only=False, hw_loop=0):
    import contextlib

    import concourse.mybir as mybir
    from concourse import bacc, tile

    f32 = mybir.dt.float32
    bf16 = mybir.dt.bfloat16
    nc = bacc.Bacc(None, target_bir_lowering=False)
    hm = nc.declare_dram_parameter("hm", [_T, _NP * _PW], bf16, isOutput=False)
    gram = nc.declare_dram_parameter(
        "gram", [128, _NPG * _GCOL], f32, isOutput=True
    )

    with tile.TileContext(nc) as tc:
        with (
            tc.tile_pool(name="hm", bufs=4) as hpool,
            tc.tile_pool(name="ps", bufs=8, space="PSUM") as ppool,
            tc.tile_pool(name="out", bufs=3) as opool,
        ):
            loop_cm = (
                tc.For_i(0, hw_loop, 1,
                         hint_engines=(mybir.EngineType.PE,))
                if hw_loop else contextlib.nullcontext()
            )
            with loop_cm:
                for _rep in range(reps):
                    slice_tiles = {}

                    def get_slice(pair):
                        s = next(
                            i for i, (p0, npr) in enumerate(_SLICES)
                            if p0 <= pair < p0 + npr
                        )
                        if s not in slice_tiles:
                            p0, npr = _SLICES[s]
                            t = hpool.tile(
                                [128, _KC, _PSMAX * _PW], bf16, tag="hm"
                            )
                            nc.sync.dma_start(
                                t[:, :, :npr * _PW],
                                hm[:, p0 * _PW:(p0 + npr) * _PW].rearrange(
                                    "(kc p) c -> p kc c", p=128
                                ),
                            )
                            slice_tiles[s] = (t, p0)
                        return slice_tiles[s]

                    ostage = None
                    for pg in range(_NPG):
                        g0 = pg * _PG
                        ng = min(_PG, _NP - g0)
                        nstrips = (ng + _SLOTS - 1) // _SLOTS
                        # one PSUM bank per strip: accumulation groups stay
                        # sequential within each bank while the PE
                        # round-robins strips (LDW overlaps MM streaming)
                        pts = [
                            ppool.tile([128, _GCOL], f32, tag="ps",
                                       name=f"pt{pg}_{j}")
                            for j in range(nstrips)
                        ]
                        if not dma_only:
                            for s in range(_SLOTS):
                                for kc in range(_KC):
                                    for j in range(nstrips):
                                        q = j * _SLOTS + s
                                        if q >= ng:
                                            continue
                                        p = g0 + q
                                        ht, sp0 = get_slice(p)
                                        c = (p - sp0) * _PW
                                        nc.tensor.matmul(
                                            pts[j][32 * j:32 * (j + 1),
                                                   s * _PW:(s + 1) * _PW],
                                            ht[:, kc, c:c + _ROWS],
                                            ht[:, kc, c:c + _PW],
                                            start=(kc == 0),
                                            stop=(kc == _KC - 1),
                                            tile_position=(0, 32 * j),
                                        )
                        if pg % _OSTG == 0:
                            ostage = opool.tile(
                                [128, _OSTG * _GCOL], f32, tag="o"
                            )
                            o0 = pg
                        for j in range(nstrips):
                            nq = min(_SLOTS, ng - j * _SLOTS)
                            rows = slice(32 * j, 32 * (j + 1))
                            od = ostage[rows, (pg - o0) * _GCOL:][:, :nq * _PW]
                            if dma_only:
                                src, _ = get_slice(g0)
                                nc.vector.tensor_copy(
                                    od, src[rows, 0, :nq * _PW]
                                )
                            else:
                                nc.vector.tensor_copy(
                                    od, pts[j][rows, :nq * _PW]
                                )
                        if pg % _OSTG == _OSTG - 1 or pg == _NPG - 1:
                            nrow = 32 * nstrips
                            ncol = _GCOL if ng >= _SLOTS else ng * _PW
                            w = (pg - o0) * _GCOL + ncol
                            nc.gpsimd.dma_start(
                                gram[:nrow, o0 * _GCOL:][:, :w],
                                ostage[:nrow, :w],
                            )

    nc.compile()
    return nc


def _prep_hm(x2):
    """x2: [T, F*CH] fp32 -> pair-packed {H|H|M|M} bf16 [T, NP*PW]."""
    import ml_dtypes

    bf = ml_dtypes.bfloat16
    H = x2.astype(bf)
    Mv = (x2 - H.astype(np.float32)).astype(bf)
    H = H.reshape(_T, _F, _CH)
    Mv = Mv.reshape(_T, _F, _CH)
    hm = np.zeros((_T, _NP, 4, _CH), dtype=bf)
    hm[:, :, 0, :] = H[:, 0::2]
    hm[:, : _F // 2, 1, :] = H[:, 1::2]
    hm[:, :, 2, :] = Mv[:, 0::2]
    hm[:, : _F // 2, 3, :] = Mv[:, 1::2]
    return hm.reshape(_T, _NP * _PW)


def _decode_gram(g):
    """g: [B, 128, NPG*GCOL] fp32 -> C [B, F, 16, 16] (~= X^T X per freq)."""
    nb = g.shape[0]
    # [B, strip(4), 32, group(9), slot(8), 64] -> pair index = (g, j, s)
    g = g.reshape(nb, _NSTRIP, _ROWS, _NPG, _SLOTS, _PW)
    g = g.transpose(0, 3, 1, 4, 2, 5).reshape(nb, _NPG * _PG, _ROWS, _PW)
    g = g[:, :_NP]
    # pair block: [H0|H1]^T [H0|H1|M0|M1]
    G1a = g[:, :, :_CH, 0 * _CH:1 * _CH]          # H0^T H0
    G1b = g[:, :, _CH:, 1 * _CH:2 * _CH]          # H1^T H1
    G2a = g[:, :, :_CH, 2 * _CH:3 * _CH]          # H0^T M0
    G2b = g[:, :, _CH:, 3 * _CH:4 * _CH]          # H1^T M1
    C = np.empty((nb, 2 * _NP, _CH, _CH), dtype=np.float32)
    C[:, 0::2] = G1a + G2a + G2a.transpose(0, 1, 3, 2)
    C[:, 1::2] = G1b + G2b + G2b.transpose(0, 1, 3, 2)
    return C[:, :_F]


def kernel(Xs):
    global _nc_cache
    from concurrent.futures import ThreadPoolExecutor

    from concourse.bass_utils import run_bass_kernel_spmd

    Xs = np.asarray(Xs, dtype=np.float32)
    assert Xs.shape == (_B, _T, _F, 2, _M)
    if _nc_cache is None:
        _nc_cache = _build_nc()

    xs2 = Xs.reshape(_B, _T, _F * _CH)
    with ThreadPoolExecutor(_B) as ex:
        hms = list(ex.map(_prep_hm, [xs2[b] for b in range(_B)]))
    in_maps = [{"hm": hms[b]} for b in range(_B)]
    res = run_bass_kernel_spmd(_nc_cache, in_maps, list(range(_NCORES))).results

    C = _decode_gram(np.stack([r["gram"] for r in res]))
    iu0, iu1 = np.triu_indices(_M)
    re = C[:, :, iu0, iu1] + C[:, :, _M + iu0, _M + iu1]
    im = C[:, :, iu0, _M + iu1] - C[:, :, iu1, _M + iu0]
    mean = np.stack([re, im], axis=2) * np.float32(1.0 / _T)  # [B, F, 2, 36]
    mean = np.ascontiguousarray(mean, dtype=np.float32)
    npairs = _M * (_M + 1) // 2
    return np.broadcast_to(mean[:, None], (_B, _T, _F, 2, npairs))



# revision 2
# speedup vs baseline: 1.0570x; 1.0570x over previous
"""Trainium2 Bass kernel for nn_Covariance — fp8 e4m3 DoubleRow, M=64.

PE-instruction-minimized variant: HW measurement showed the PE serializes
LDWEIGHTS and matmul streams (~0.5 ns/col combined) with a ~15-25 ns
per-instruction overhead, so the kernel packs TWO frequency pairs per
matmul (stationary = 64 channels of 4 frequencies, DoubleRow K=256):
129 two-pair blocks x 2 chunks = 258 ldweights+matmul pairs total
(vs 1028 for the 32-wide single-rate variant).  Only the diagonal 32x32
pair-Grams of each 64x64 block are copied out (strided slot copies), so
the bf16 output stays at 0.53 MB.  DoubleRow is ISA-restricted to array
position (0,0) / PSUM partition offset 0; four PSUM banks rotate to keep
accumulation groups legal.  Input layout is sub-block-major: each
(group, slot-pair) is one contiguous [128 x 2 KB] = 256 KB DMA.

Per core: read 4.23 MB (fp8), write 0.53 MB (bf16 Gram diagonals).
"""

import numpy as np

_B, _T, _F, _M = 8, 512, 513, 8
_CH = 2 * _M            # 16 packed re/im channels
_PW = 2 * _CH           # 32 fp8 columns per frequency pair
_BW = 2 * _PW           # 64 columns per two-pair block
_NP = (_F + 1) // 2     # 257 frequency pairs
_NB = (_NP + 1) // 2    # 129 two-pair blocks (pair 257 zero-padded)
_KC = 4                 # SBUF time chunks of 128 (DoubleRow takes 2)
_NCORES = 8
_NBANK = 4              # PSUM banks rotated
_SLOTS = 8              # blocks per bank ([128, 8, 64] f32 = one bank)
_SB = 2                 # slots per sub-DMA block
_BG = _NBANK * _SLOTS   # 32 blocks per PSUM group
_NPG = (_NB + _BG - 1) // _BG   # 5 groups (last holds 1 block)
_GRP = [(g * _BG, min(_BG, _NB - g * _BG)) for g in range(_NPG)]
# sub-blocks: (group, sb) -> (col offset, nbank, nslot_in_sb, size)
_SUB = []
_off = 0
for _g, (_b0, _nbg) in enumerate(_GRP):
    _nbank = (_nbg + _SLOTS - 1) // _SLOTS
    _nsl = min(_SLOTS, _nbg)
    for _sb in range((_nsl + _SB - 1) // _SB):
        _ns = min(_SB, _nsl - _sb * _SB)
        _sz = _KC * _nbank * _ns * _BW
        _SUB.append((_g, _sb, _off, _nbank, _ns, _sz))
        _off += _sz
_TOTCOL = _off          # fp8 bytes per partition row
_OCOL = [min(nbg, _BG) * _PW for _b0, nbg in _GRP]   # 32 gram cols per block
_OOFF = np.cumsum([0] + _OCOL).tolist()

_nc_cache = None


def _f8():
    import ml_dtypes

    return ml_dtypes.float8_e4m3


def _build_nc(reps=1, hw_loop=0, mode="full"):
    import contextlib

    import concourse.mybir as mybir
    from concourse import bacc, tile

    f32 = mybir.dt.float32
    bf16 = mybir.dt.bfloat16
    fp8 = mybir.dt.float8e4
    DR = mybir.MatmulPerfMode.DoubleRow
    nc = bacc.Bacc(None, target_bir_lowering=False)
    hm = nc.declare_dram_parameter("hm", [128, _TOTCOL], fp8, isOutput=False)
    gram = nc.declare_dram_parameter(
        "gram", [_BW, _OOFF[-1]], bf16, isOutput=True
    )
    import re as _re

    _m = _re.match(r"([a-z]+)(\d*)$", mode)
    mode, hbufs = _m.group(1), int(_m.group(2) or 16)
    do_dma = mode in ("full", "nomm")
    do_mm = mode in ("full", "mm")

    with tile.TileContext(nc) as tc:
        with (
            tc.tile_pool(name="hm", bufs=hbufs) as hpool,
            tc.tile_pool(name="ps", bufs=8, space="PSUM") as ppool,
            tc.tile_pool(name="out", bufs=3) as opool,
        ):
            loop_cm = (
                tc.For_i(0, hw_loop, 1,
                         hint_engines=(mybir.EngineType.PE,))
                if hw_loop else contextlib.nullcontext()
            )
            with loop_cm:
                for _rep in range(reps):
                    si = 0
                    for pg in range(_NPG):
                        b0, nbg = _GRP[pg]
                        nbank = (nbg + _SLOTS - 1) // _SLOTS
                        pts = [
                            ppool.tile([128, _SLOTS, _BW], f32, tag="ps",
                                       name=f"pt{pg}_{j}")
                            for j in range(nbank)
                        ] if do_mm else []
                        hts = []
                        nsl = min(_SLOTS, nbg)
                        for sb in range((nsl + _SB - 1) // _SB):
                            g_, sb_, off, nbk, ns, sz = _SUB[si]
                            assert (g_, sb_) == (pg, sb)
                            si += 1
                            # [128, kc, bank, slot*64ch] contiguous block
                            ht = hpool.tile(
                                [128, _KC, _NBANK, _SB * _BW], fp8, tag="hm"
                            )
                            hts.append(ht)
                            if do_dma:
                                nc.sync.dma_start(
                                    ht[:, :, :nbk, :ns * _BW],
                                    hm[:, off:off + sz].rearrange(
                                        "p (kc j c) -> p kc j c",
                                        kc=_KC, j=nbk,
                                    ),
                                )
                            else:
                                # sliver write so the tile counts as produced
                                nc.sync.dma_start(
                                    ht[:, 0, 0, :_BW],
                                    hm[:, off:off + _BW],
                                )
                            if do_mm:
                                for s2 in range(ns):
                                    s = sb * _SB + s2
                                    for c in range(2):
                                        for j in range(nbank):
                                            if j * _SLOTS + s >= nbg:
                                                continue
                                            cc = s2 * _BW
                                            nc.tensor.matmul(
                                                pts[j][:_BW, s, :],
                                                ht[:, 2 * c:2 * c + 2, j,
                                                   cc:cc + _BW],
                                                ht[:, 2 * c:2 * c + 2, j,
                                                   cc:cc + _BW],
                                                start=(c == 0),
                                                stop=(c == 1),
                                                perf_mode=DR,
                                            )
                        ostage = opool.tile([_BW, _BG, _PW], bf16, tag="o")
                        for j in range(nbank):
                            nq = min(_SLOTS, nbg - j * _SLOTS)
                            for h in range(2):  # the two diagonal 32x32s
                                rows = slice(_PW * h, _PW * (h + 1))
                                dst = ostage[rows,
                                             j * _SLOTS:j * _SLOTS + nq, :]
                                if do_mm:
                                    src = pts[j][rows, :nq,
                                                 h * _PW:(h + 1) * _PW]
                                elif nq >= 2:
                                    # consume the DMA'd tile instead
                                    src = hts[0][rows, :nq // 2, 0,
                                                 :_BW].rearrange(
                                        "p a (b c) -> p a b c", c=_PW
                                    )
                                    dst = dst.rearrange(
                                        "p (a b) c -> p a b c", b=2
                                    )
                                else:
                                    src = hts[0][rows, 0, 0, :_PW]
                                    dst = ostage[rows, j * _SLOTS, :]
                                if (j + h) % 2 == 0:
                                    nc.vector.tensor_copy(dst, src)
                                else:
                                    nc.scalar.copy(dst, src)
                        nblk = min(nbg, _BG)
                        nc.gpsimd.dma_start(
                            gram[:, _OOFF[pg]:_OOFF[pg + 1]].rearrange(
                                "p (s c) -> p s c", c=_PW
                            ),
                            ostage[:, :nblk, :],
                        )

    nc.compile()
    return nc


def _prep_hm(x2):
    """x2: [T, F*CH] fp32 -> sub-block-packed e4m3 [128, TOTCOL]."""
    f8 = _f8()
    q = x2.astype(f8).reshape(_T, _F, _CH)
    hmp = np.zeros((_T, 2 * _NB, 2, _CH), dtype=f8)
    hmp[:, :_NP, 0, :] = q[:, 0::2]
    hmp[:, : _F // 2, 1, :] = q[:, 1::2]
    a = hmp.reshape(_KC, 128, _NB, _BW)      # t = kc*128 + p
    out = np.empty((128, _TOTCOL), dtype=f8)
    for g_, sb_, off, nbk, ns, sz in _SUB:
        b0 = _GRP[g_][0]
        idx = [
            b0 + j * _SLOTS + sb_ * _SB + s2
            for j in range(nbk) for s2 in range(ns)
        ]
        blk = a[:, :, idx, :]                # [KC, 128, nbk*ns, BW]
        out[:, off:off + sz] = blk.transpose(1, 0, 2, 3).reshape(128, -1)
    return out


def _decode_gram(g):
    """g: [B, 64, OOFF[-1]] bf16 -> C [B, F, 16, 16]."""
    nb_ = g.shape[0]
    g = np.asarray(g, dtype=np.float32)
    pair_blocks = []
    for pg, (b0, nbg) in enumerate(_GRP):
        seg = g[:, :, _OOFF[pg]:_OOFF[pg + 1]]
        nblk = min(nbg, _BG)
        seg = seg.reshape(nb_, 2, _PW, nblk, _PW)
        # block index within group = (j*SLOTS + s) in storage order;
        # halves h are the two pairs (2k, 2k+1)
        seg = seg.transpose(0, 3, 1, 2, 4)   # [B, blk, h, 32, 32]
        pair_blocks.append(seg.reshape(nb_, -1, _PW, _PW))
    g = np.concatenate(pair_blocks, axis=1)[:, :_NP]
    C = np.empty((nb_, 2 * _NP, _CH, _CH), dtype=np.float32)
    C[:, 0::2] = g[:, :, :_CH, :_CH]
    C[:, 1::2] = g[:, :, _CH:, _CH:]
    return C[:, :_F]


def kernel(Xs):
    global _nc_cache
    from concurrent.futures import ThreadPoolExecutor

    from concourse.bass_utils import run_bass_kernel_spmd

    Xs = np.asarray(Xs, dtype=np.float32)
    assert Xs.shape == (_B, _T, _F, 2, _M)
    if _nc_cache is None:
        _nc_cache = _build_nc()

    xs2 = Xs.reshape(_B, _T, _F * _CH)
    with ThreadPoolExecutor(_B) as ex:
        hms = list(ex.map(_prep_hm, [xs2[b] for b in range(_B)]))
    in_maps = [{"hm": hms[b]} for b in range(_B)]
    res = run_bass_kernel_spmd(_nc_cache, in_maps, list(range(_NCORES))).results

    C = _decode_gram(np.stack([r["gram"] for r in res]))
    iu0, iu1 = np.triu_indices(_M)
    re = C[:, :, iu0, iu1] + C[:, :, _M + iu0, _M + iu1]
    im = C[:, :, iu0, _M + iu1] - C[:, :, iu1, _M + iu0]
    mean = np.stack([re, im], axis=2) * np.float32(1.0 / _T)  # [B, F, 2, 36]
    mean = np.ascontiguousarray(mean, dtype=np.float32)
    npairs = _M * (_M + 1) // 2
    return np.broadcast_to(mean[:, None], (_B, _T, _F, 2, npairs))
